# revision 3
# baseline (speedup 1.0000x reference)
"""Differential attention kernel for TRN2, 8 NeuronCores — v2.

Problem: q,k,v [2, 2048, 8, 64] f32; out [2, 8, 1024, 64]:
  S = (Q @ K^T) / 8 per (b,h); P = softmax(S); out = (P[:1024] - lam*P[1024:]) @ V
  lam = exp(lq1.lk1) - exp(lq2.lk2) + LAMBDA_INIT  (computed on host)

Sharding: 16 (b,h) slabs, 2 per core.

v2 design (engine-measured on this hardware):
  - ACT exp is the binding engine (~1.16ns/row for f16 out); everything else
    is scheduled to hide under the 64 x [128,1024] exp instructions.
  - QK: contraction dim padded 64->128 with zeros (k<128 matmuls run at half
    rate on TRN2); Q^T/K^T arenas are f32 tiles bitcast to f32r for the PE.
    S^T_j chunks [128 keys, 512 q] at ~246ns each.
  - PV: ES chunk [128k, 128q] f16 as the *stationary* operand, V'_j [128k, 65]
    f16 moving (ones in col 64 fold in the softmax denominator). Output
    accumulates in natural [q, d] layout -> no PE transposes, 65-row matmuls
    at ~30ns.
  - PSUM: st [128,1024]x2 bufs (4 banks) + O [128,2,4,128]x2 bufs (4 banks),
    each O q-slice at a 512B offset so accumulation regions never cross banks.
  - Pool engine zeroes the pad rows of the arenas; DVE does conversions and
    the normalize/combine epilogue; lambda arrives host-computed, replicated
    to [128,1].
"""

import math
import sys

sys.path.insert(0, "/opt/trn_rl_repo")

import os
import numpy as np

NO_PV = bool(int(os.environ.get("K2_NO_PV", "0")))

B, N, H, D = 2, 2048, 8, 64
P = 128
NT = N // P  # 16 key tiles per slab
NH = NT // 2  # 8 query tiles per half
SLABS_PER_CORE = 2
N_CORES = 8
LAMBDA_INIT = 0.8 - 0.6 * math.exp(-0.3 * 0.8)

_cached_nc = {}


def _build_program(repeats=1):
    if repeats in _cached_nc:
        return _cached_nc[repeats]

    import concourse.mybir as mybir
    import concourse.tile as tile
    from concourse import bacc

    f32 = mybir.dt.float32
    f32r = mybir.dt.float32r
    f16 = mybir.dt.float16
    i32 = mybir.dt.int32
    AF = mybir.ActivationFunctionType
    ALU = mybir.AluOpType
    EXP_A = float(2.0**23 / math.log(2.0))
    EXP_B = 127.0 * 2.0**23 - 486411.0
    DVE_J = {8, 11, 14}  # exp steps offloaded to DVE (Schraudolph bit-trick)

    nc = bacc.Bacc("TRN2", target_bir_lowering=False, debug=False)
    qs = nc.dram_tensor("qs", [SLABS_PER_CORE, D, N], f32, kind="ExternalInput").ap()
    ks = nc.dram_tensor("ks", [SLABS_PER_CORE, D, N], f32, kind="ExternalInput").ap()
    vs = nc.dram_tensor("vs", [SLABS_PER_CORE, N, D], f32, kind="ExternalInput").ap()
    lam = nc.dram_tensor("lam", [P, 1], f32, kind="ExternalInput").ap()
    out = nc.dram_tensor(
        "out", [SLABS_PER_CORE, N // 2, D], f32, kind="ExternalOutput"
    ).ap()

    with tile.TileContext(nc) as tc:
        with (
            tc.tile_pool(name="const", bufs=1) as cpool,
            tc.tile_pool(name="inp", bufs=2) as inpool,
            tc.tile_pool(name="tq", bufs=2) as tqpool,
            tc.tile_pool(name="es", bufs=6) as espool,
            tc.tile_pool(name="esi", bufs=2) as esipool,
            tc.tile_pool(name="fin", bufs=2) as finpool,
            tc.tile_pool(name="pst", bufs=3, space="PSUM") as pstpool,
            tc.tile_pool(name="po", bufs=1, space="PSUM") as popool,
        ):
            lamt = cpool.tile([P, 1], f32)
            nc.sync.dma_start(lamt[:], lam)
            onescol = cpool.tile([P, NT], f32)
            nc.vector.memset(onescol[:], 1.0)
            zrow = cpool.tile([P - D, N], f32)
            nc.vector.memset(zrow[:], 0.0)

            def emit_loads(s_rep):
                """DMAs + conversion thunks for one slab. Returns
                (tiles, upfront_thunks, filler_thunks)."""
                s = s_rep % SLABS_PER_CORE
                qf = inpool.tile([D, N], f32, tag="qf")
                kf = inpool.tile([D, N], f32, tag="kf")
                vn = inpool.tile([P, NT, D], f32, tag="vn")
                nc.sync.dma_start(kf[:, 0:512], ks[s][:, 0:512])
                nc.sync.dma_start(qf[:, 0:1024], qs[s][:, 0:1024])
                nc.sync.dma_start(vn[:], vs[s].rearrange("(t p) d -> p t d", p=P))
                nc.sync.dma_start(kf[:, 512:N], ks[s][:, 512:N])
                nc.sync.dma_start(qf[:, 1024:N], qs[s][:, 1024:N])

                # arenas: [128, ...] with rows 64..127 zeroed on the Pool
                # engine (k=128 matmuls run 2x faster than k=64)
                qt = tqpool.tile([P, N], f32r, tag="qt")
                kt = tqpool.tile([P, NT, P], f32r, tag="kt")
                va = tqpool.tile([P, NT, D + 1], f16, tag="va")

                def mz_kt():
                    nc.gpsimd.tensor_copy(
                        kt[D:P, :, :], zrow[:].rearrange("p (t q) -> p t q", q=P)
                    )

                def mz_qt():
                    nc.gpsimd.tensor_copy(qt[D:P, :], zrow[:])

                def conv_va():
                    nc.vector.tensor_copy(va[:, :, 0:D], vn[:])
                    nc.vector.tensor_copy(va[:, :, D], onescol[:])

                def conv_kt(c0, nch=1):
                    def thunk():
                        nc.vector.tensor_copy(
                            kt[0:D, 4 * c0 : 4 * (c0 + nch), :],
                            kf[:, 512 * c0 : 512 * (c0 + nch)].rearrange(
                                "d (t p) -> d t p", p=P
                            ),
                        )

                    return thunk

                def conv_qt(c0, nch=1):
                    def thunk():
                        nc.vector.tensor_copy(
                            qt[0:D, 512 * c0 : 512 * (c0 + nch)],
                            qf[:, 512 * c0 : 512 * (c0 + nch)],
                        )

                    return thunk

                upfront = [mz_kt, mz_qt, conv_kt(0), conv_qt(0, 2), conv_va]
                fillers = [conv_kt(1), conv_kt(2), conv_kt(3), conv_qt(2), conv_qt(3)]
                return (qt, kt, va), upfront, fillers

            def emit_half(qt, kt, va, ih, fillers):
                """One query-half j-loop. Returns the O psum tile + its
                epilogue thunk (drain O to SBUF)."""
                o = popool.tile([P, 2, 4, P], f32, tag="o", name=f"o_{ih}")
                pending = None
                for j in range(NT):
                    st = pstpool.tile([P, 2 * 512], f32, tag="st")
                    es = espool.tile([P, 2 * 512], f16, tag="es")
                    for c in range(2):
                        nc.tensor.matmul(
                            st[:, c * 512 : (c + 1) * 512],
                            kt[:, j, :],
                            qt[:, N // 2 * ih + 512 * c : N // 2 * ih + 512 * (c + 1)],
                            start=True,
                            stop=True,
                        )
                    if j in DVE_J:
                        ii = esipool.tile([P, 2 * 512], i32, tag="esi")
                        nc.vector.tensor_scalar(
                            ii[:], st[:], EXP_A, EXP_B, ALU.mult, ALU.add
                        )
                        nc.vector.tensor_copy(es[:], ii[:].bitcast(f32))
                    else:
                        nc.scalar.activation(es[:], st[:], AF.Exp, scale=1.0)
                    if NO_PV:
                        pending = (j, es)
                        if fillers:
                            fillers.pop(0)()
                        continue
                    if pending is not None:
                        jp, esp = pending
                        for qc in range(8):
                            # PSUM start zeroes the whole bank: only the first
                            # region per bank starts; siblings accumulate onto
                            # the freshly zeroed bank
                            nc.tensor.matmul(
                                o[:, qc // 4, qc % 4, 0 : D + 1],
                                esp[:, P * qc : P * (qc + 1)],
                                va[:, jp, :],
                                start=(jp == 0 and qc % 4 == 0),
                                stop=(jp == NT - 1),
                                skip_group_check=True,
                            )
                    pending = (j, es)
                    if fillers:
                        fillers.pop(0)()
                if not NO_PV:
                    jp, esp = pending
                    for qc in range(8):
                        nc.tensor.matmul(
                            o[:, qc // 4, qc % 4, 0 : D + 1],
                            esp[:, P * qc : P * (qc + 1)],
                            va[:, jp, :],
                            start=(jp == 0 and qc % 4 == 0),
                            stop=(jp == NT - 1),
                            skip_group_check=True,
                        )
                else:
                    nc.tensor.matmul(
                        o[:, 0, 0, 0 : D + 1],
                        es[:, 0:P],
                        va[:, 0, :],
                        start=True,
                        stop=True,
                        skip_group_check=True,
                    )

                onn = finpool.tile([P, 2, 4, D + 1], f32, tag=f"onn{ih}", name=f"onn{ih}")

                def epilogue():
                    nc.vector.tensor_copy(onn[:], o[:, :, :, 0 : D + 1])

                return onn, epilogue

            def emit_combine(s_rep, onn0, onn1):
                """Normalize both halves, apply lambda, subtract, DMA out."""
                s = s_rep % SLABS_PER_CORE

                def thunk():
                    rec = finpool.tile([P, 2, 2, 4], f32, tag="rec")
                    nc.vector.reciprocal(rec[:, 0], onn0[:, :, :, D])
                    nc.vector.reciprocal(rec[:, 1], onn1[:, :, :, D])
                    nc.vector.tensor_scalar_mul(rec[:, 1], rec[:, 1], lamt[:, 0:1])
                    res = finpool.tile([P, 2, 2, 4, D], f32, tag="res")
                    nc.vector.tensor_mul(
                        res[:, 0],
                        onn0[:, :, :, 0:D],
                        rec[:, 0].broadcast_to([P, 2, 4, D]),
                    )
                    nc.vector.tensor_mul(
                        res[:, 1],
                        onn1[:, :, :, 0:D],
                        rec[:, 1].broadcast_to([P, 2, 4, D]),
                    )
                    dd = finpool.tile([P, 2, 4, D], f32, tag="dd")
                    nc.vector.tensor_sub(dd[:], res[:, 0], res[:, 1])
                    nc.sync.dma_start(
                        out[s].rearrange("(t p) d -> p t d", p=P),
                        dd[:].rearrange("p a u d -> p (a u) d"),
                    )

                return thunk

            def emit_all():
                tiles, upfront, fillers = emit_loads(0)
                for thunk in upfront:
                    thunk()
                pending = fillers
                for s_rep in range(SLABS_PER_CORE):
                    qt, kt, va = tiles
                    onn0, epi0 = emit_half(qt, kt, va, 0, pending)
                    if s_rep + 1 < SLABS_PER_CORE:
                        tiles, nxt_up, nxt_fill = emit_loads(s_rep + 1)
                    else:
                        tiles, nxt_up, nxt_fill = None, [], []
                    pending = [epi0] + nxt_up
                    onn1, epi1 = emit_half(qt, kt, va, 1, pending)
                    comb = emit_combine(s_rep, onn0, onn1)
                    if s_rep + 1 < SLABS_PER_CORE:
                        # interleave leftovers + this slab's tail into the
                        # next slab's ih0 loop
                        pending = (
                            pending
                            + [nxt_fill[0], epi1, nxt_fill[1], comb]
                            + nxt_fill[2:]
                        )
                    else:
                        for thunk in pending:
                            thunk()
                        epi1()
                        comb()
                        pending = []

            if repeats == 1:
                emit_all()
            else:
                with tc.For_i(0, repeats, 1):
                    emit_all()

    nc.compile()
    _cached_nc[repeats] = nc
    return nc


def make_in_maps(inputs):
    q = np.asarray(inputs["q"], dtype=np.float32)
    k = np.asarray(inputs["k"], dtype=np.float32)
    v = np.asarray(inputs["v"], dtype=np.float32)
    l1 = float(
        np.exp(
            np.sum(
                np.asarray(inputs["lambda_q1"], np.float64)
                * np.asarray(inputs["lambda_k1"], np.float64)
            )
        )
    )
    l2 = float(
        np.exp(
            np.sum(
                np.asarray(inputs["lambda_q2"], np.float64)
                * np.asarray(inputs["lambda_k2"], np.float64)
            )
        )
    )
    lam_full = np.float32(l1 - l2 + LAMBDA_INIT)
    lamr = np.full((P, 1), lam_full, dtype=np.float32)

    qs = np.ascontiguousarray(q.transpose(0, 2, 3, 1) * 0.125).reshape(B * H, D, N)
    ks = np.ascontiguousarray(k.transpose(0, 2, 3, 1)).reshape(B * H, D, N)
    vs = np.ascontiguousarray(v.transpose(0, 2, 1, 3)).reshape(B * H, N, D)
    return [
        {
            "qs": qs[SLABS_PER_CORE * c : SLABS_PER_CORE * (c + 1)],
            "ks": ks[SLABS_PER_CORE * c : SLABS_PER_CORE * (c + 1)],
            "vs": vs[SLABS_PER_CORE * c : SLABS_PER_CORE * (c + 1)],
            "lam": lamr,
        }
        for c in range(N_CORES)
    ]


def kernel(q, k, v, lambda_q1, lambda_k1, lambda_q2, lambda_k2, **_unused):
    from concourse.bass_utils import run_bass_kernel_spmd

    in_maps = make_in_maps(
        dict(
            q=q, k=k, v=v,
            lambda_q1=lambda_q1, lambda_k1=lambda_k1,
            lambda_q2=lambda_q2, lambda_k2=lambda_k2,
        )
    )
    nc = _build_program()
    res = run_bass_kernel_spmd(nc, in_maps, core_ids=list(range(N_CORES)))
    outs = np.stack([res.results[c]["out"] for c in range(N_CORES)])
    return outs.reshape(B, H, N // 2, D).astype(np.float32)
